# revision 11
# baseline (speedup 1.0000x reference)
"""Channel Attention Module (CAM) TRN2 Bass kernel.

Reference (per batch b of x[B, H, W, C]):
    a    = x[b].reshape(HW, C)
    G    = a.T @ a                      # [C, C]
    attn = softmax(G, axis=-1)
    out  = gamma * (a @ attn) + x[b]

Sharding: data-parallel over batch. B=16 across 8 cores -> 2 batches/core.
No cross-core communication.

Per-core schedule (fp32 end to end):
  stage A (per 128-row chunk c of a): accumulate gram via 2 matmuls
    (stationary = a-chunk channel halves), and transpose the chunk with 2
    more matmuls against identity (out = lhsT.T @ I), giving aT in SBUF.
  stage B: row softmax of G in PSUM: reduce_max(negate) -> exp with
    per-partition bias and fused row-sum -> reciprocal -> scale by
    (1/s * gamma)  (gamma folded into attn so the epilogue is a plain add).
  stage C (per chunk): psum_O = aT_chunk.T @ attn (2 accumulating matmuls),
    out = psum_O + x_chunk, staged into groups of 8 chunks per output DMA.
"""

import numpy as np

P = 128
C = 256
HW = 4096
NCH = HW // P          # 32 n-chunks per batch
BPC = 2                # batches per core
GRP = 8                # chunks per output DMA group
N_CORES = 8


def _split_excess_waits(nc, max_waits=1):
    """walrus on this toolchain rejects >1 semaphore wait on one
    instruction ("Too many sync wait commands"); split extras into
    wait-only Drain instructions inserted just before."""
    import concourse.mybir as mybir

    n_split = 0
    for fn in nc.m.functions:
        for b in fn.blocks:
            insts = b.instructions
            i = 0
            while i < len(insts):
                inst = insts[i]
                si = inst.sync_info
                if si is not None and si.on_wait and len(si.on_wait) > max_waits:
                    waits = list(si.on_wait)
                    si_t = type(si)
                    n_extra = (len(waits) - 1) // max_waits
                    extra = []
                    for j in range(n_extra):
                        d = mybir.InstDrain(
                            name=nc.get_next_instruction_name(),
                            ins=[], outs=[], bass_is_fusable=False,
                        )
                        d.engine = inst.engine
                        d.sync_info = si_t(
                            on_wait=waits[j * max_waits:(j + 1) * max_waits],
                            on_update=[],
                        )
                        extra.append(d)
                    inst.sync_info = si_t(
                        on_wait=waits[n_extra * max_waits:],
                        on_update=si.on_update,
                    )
                    for k, d in enumerate(extra):
                        insts.insert(i + k, d)
                        nc.register_instruction(d)
                    i += n_extra
                    n_split += 1
                i += 1
    return n_split


def _soften_psum_war_drains(nc):
    """Tile guards PSUM-slot reuse (WAR) with a Drain+wait on the PE
    sequencer; the drain empties the PE pipe every chunk, which both
    serializes dispatch and keeps HAM clock-gated at 1.2 GHz.  A plain
    dispatch-level wait is sufficient for the WAR hazard (the consumer's
    semaphore increments at completion and the PE executes in order), so
    convert wait-only drains in the main body into NoOps carrying the
    same wait."""
    from concourse import mybir

    n = 0
    for fn in nc.m.functions:
        for b in fn.blocks:
            if b.name.endswith("_end"):
                continue
            insts = b.instructions
            for idx, inst in enumerate(insts):
                if isinstance(inst, mybir.InstDrain):
                    si = inst.sync_info
                    if si is not None and si.on_wait and not si.on_update:
                        nop = mybir.InstNoOp(
                            name=inst.name,
                            engine=inst.engine,
                            bass_nofuse=True,
                            sync_info=si,
                        )
                        insts[idx] = nop
                        nc.register_instruction(nop, overwrite=True)
                        n += 1
    return n


def _build():
    import concourse.bass as bass
    import concourse.tile as tile
    from concourse import mybir
    from concourse.masks import make_identity

    f32 = mybir.dt.float32
    bf16 = mybir.dt.bfloat16
    nc = bass.Bass("TRN2", target_bir_lowering=False, debug=False)

    x_ext = nc.declare_dram_parameter("x", [BPC, HW, C], f32, isOutput=False)
    g_ext = nc.declare_dram_parameter("gamma", [1], f32, isOutput=False)
    out_ext = nc.declare_dram_parameter("out", [BPC, HW, C], f32, isOutput=True)

    with tile.TileContext(nc) as tc:
        with (
            tc.tile_pool(name="const", bufs=1) as const_pool,
            tc.tile_pool(name="a", bufs=2) as a_pool,
            tc.tile_pool(name="abf", bufs=2) as abf_pool,
            tc.tile_pool(name="at", bufs=2) as at_pool,
            tc.tile_pool(name="attn", bufs=2) as attn_pool,
            tc.tile_pool(name="small", bufs=2) as small_pool,
            tc.tile_pool(name="outs", bufs=3) as out_pool,
            tc.tile_pool(name="psG", bufs=2, space="PSUM") as psG_pool,
            tc.tile_pool(name="psT", bufs=3, space="PSUM") as psT_pool,
            tc.tile_pool(name="psO", bufs=3, space="PSUM") as psO_pool,
        ):
            ident = const_pool.tile([P, P], bf16)
            make_identity(nc, ident[:])

            # gamma -> all 128 partitions (step-0 DMA broadcast)
            gamma_bc = const_pool.tile([P, 1], f32)
            nc.sync.dma_start(gamma_bc[:], g_ext[None, :].to_broadcast((P, 1)))

            for b in range(BPC):
                xr = x_ext[b].rearrange("(c p) f -> p c f", p=P)
                outr = out_ext[b].rearrange("(c p) f -> p c f", p=P)

                a_sb = a_pool.tile([P, NCH, C], f32)
                a_bf = abf_pool.tile([P, NCH, C], bf16)
                at_sb = at_pool.tile([P, 2, HW], bf16)
                for g in range(4):
                    nc.sync.dma_start(
                        a_sb[:, bass.ts(g, 8), :], xr[:, bass.ts(g, 8), :]
                    )

                # ---- stage A: cast to bf16, gram accumulation, transpose ----
                psum_G = psG_pool.tile([P, 2 * C], f32)
                for c in range(NCH):
                    nc.vector.tensor_copy(a_bf[:, c, :], a_sb[:, c, :])
                    for ic in range(2):
                        nc.tensor.matmul(
                            psum_G[:, bass.ts(ic, C)],
                            a_bf[:, c, bass.ts(ic, P)],
                            a_bf[:, c, :],
                            start=(c == 0),
                            stop=(c == NCH - 1),
                        )
                    psum_T = psT_pool.tile([P, C], f32)
                    for ic in range(2):
                        nc.tensor.matmul(
                            psum_T[:, bass.ts(ic, P)],
                            a_bf[:, c, bass.ts(ic, P)],
                            ident[:],
                            start=True,
                            stop=True,
                        )
                    # psum_T[i_loc, ic*128+n_loc] -> at_sb[i_loc, ic, c*128+n_loc]
                    nc.scalar.copy(
                        at_sb[:, :, bass.ts(c, P)],
                        psum_T[:].rearrange("p (ic n) -> p ic n", ic=2),
                    )

                # ---- stage B: row softmax, gamma & 1/s folded into attn ----
                negmax = small_pool.tile([P, 2], f32)
                ssum = small_pool.tile([P, 2], f32)
                rg = small_pool.tile([P, 2], f32)
                attn = attn_pool.tile([P, 2, C], bf16)
                for ic in range(2):
                    nc.vector.reduce_max(
                        negmax[:, ic:ic + 1],
                        psum_G[:, bass.ts(ic, C)],
                        axis=mybir.AxisListType.X,
                        negate=True,
                    )
                E = attn_pool.tile([P, 2, C], f32)
                for ic in range(2):
                    nc.scalar.activation(
                        E[:, ic, :],
                        psum_G[:, bass.ts(ic, C)],
                        mybir.ActivationFunctionType.Exp,
                        bias=negmax[:, ic:ic + 1],
                        scale=1.0,
                        accum_out=ssum[:, ic:ic + 1],
                    )
                recip = small_pool.tile([P, 2], f32)
                nc.vector.reciprocal(recip[:], ssum[:])
                nc.vector.tensor_scalar_mul(rg[:], recip[:], gamma_bc[:, 0:1])
                for ic in range(2):
                    nc.vector.tensor_scalar_mul(
                        attn[:, ic, :], E[:, ic, :], rg[:, ic:ic + 1]
                    )

                # ---- stage C: out = aT.T @ attn + x ----
                for g in range(NCH // GRP):
                    out_sb = out_pool.tile([P, GRP, C], f32)
                    for cc in range(GRP):
                        c = g * GRP + cc
                        psum_O = psO_pool.tile([P, C], f32)
                        for ic in range(2):
                            nc.tensor.matmul(
                                psum_O[:],
                                at_sb[:, ic, bass.ts(c, P)],
                                attn[:, ic, :],
                                start=(ic == 0),
                                stop=(ic == 1),
                            )
                        nc.vector.tensor_tensor(
                            out_sb[:, cc, :],
                            psum_O[:],
                            a_sb[:, c, :],
                            mybir.AluOpType.add,
                        )
                    nc.sync.dma_start(
                        outr[:, bass.ts(g, GRP), :], out_sb[:]
                    )

    return nc


_NC = None


def _get_nc():
    global _NC
    if _NC is None:
        nc = _build()
        # These passes must run after _build returns: Tile's lazily
        # materialized sem-wait instructions only appear in the block
        # lists once the builder scope has fully unwound.
        _soften_psum_war_drains(nc)
        _split_excess_waits(nc)
        _NC = nc
    return _NC


def kernel(x: np.ndarray, gamma: np.ndarray) -> np.ndarray:
    from concourse.bass_utils import run_bass_kernel_spmd

    B, H, W, Cc = x.shape
    assert (B, H, W, Cc) == (16, 64, 64, 256)
    nc = _get_nc()
    xs = np.ascontiguousarray(
        x.reshape(N_CORES, BPC, HW, C).astype(np.float32, copy=False)
    )
    gamma = np.ascontiguousarray(gamma.astype(np.float32, copy=False))
    in_maps = [{"x": xs[i], "gamma": gamma} for i in range(N_CORES)]
    res = run_bass_kernel_spmd(nc, in_maps, core_ids=list(range(N_CORES)))
    out = np.stack([res.results[i]["out"] for i in range(N_CORES)])
    return out.reshape(B, H, W, Cc)
